# revision 3
# baseline (speedup 1.0000x reference)
"""Multi-head self-attention (B=4, N=2048, C=768, H=12, causal + RoPE) on 8 TRN2 cores.

Sharding: core = (batch b = core // 2, head-group g = core % 2); each core computes
6 heads of one batch end-to-end (qkv -> rope -> causal flash attention -> partial
output projection over its 384 channels). Host sums the two partial projections
per batch and adds the bias.

v4 design: single fused software-pipelined emission. The PE queue is in-order,
so the attention inner loop emits, per k-tile pair group:
    [score matmul pair(s) for group g] [one qkv/proj filler chain] [PV for g-1]
The filler chain gives the Scalar engine time to finish exp(g-1) before the PE
reaches PV(g-1) -> no head-of-line blocking, PE stays dense, and the qkv/proj
work rides in the attention phase's dependency gaps.

Other key points (vs the original baseline):
  - fp16 everywhere on SBUF (PSUM fp32); better accuracy than bf16 + DVE 2x.
  - causal: score/PV matmuls + exp restricted to the valid q-suffix on diagonal
    tiles; diagonal triangle applied post-exp as P *= mask01 (fp16 DVE 2x).
  - softmax denominator: V tiles hold [ones | V_h] per head so the PV matmul
    replicates the denominator into psum rows 0:64 for free; normalize is
    reciprocal_approx_fast (partition base 0!) + partition-shift DMA + one stt.
  - rope: stt from PSUM with fp16 cos/sin tables + free half-swap DMA + add.
  - DMA queues: SP = all mid-kernel SBUF<->SBUF moves + x loads; ACT = initial
    weight loads only (exps must never wait behind a DMA trigger); GPSIMD
    (SWDGE) = wva/proj weights + output stores.
"""

import os
import sys

import numpy as np

sys.path.insert(0, "/opt/trn_rl_repo")

import concourse.bass as bass
import concourse.mybir as mybir
import concourse.tile as tile
from concourse import bacc
from concourse.bass_utils import run_bass_kernel_spmd

dt = mybir.dt
F32 = dt.float32
F16 = dt.float16
F8 = dt.float8e4
AF = mybir.ActivationFunctionType
ALU = mybir.AluOpType

B, N, C = 4, 2048, 768
H, HD = 12, 64
HL = 6            # heads per core
G = 2             # head groups (cores per batch)
NCORES = 8
NT = N // 128     # 16 n-tiles
QB = 512          # query block
NQB = N // QB     # 4 query blocks
CT = C // 128     # 6 contraction tiles of x channels
DL = HL * HD      # 384 local channels
SCALE = float(HD) ** -0.5
PV_FP8 = bool(int(os.environ.get("PV_FP8", "0")))
# fp8 P: exp(s*SCALE - EXP_BIAS): shifts the (unnormalized) softmax weights
# into e4m3 range (max observed s*SCALE ~ 6.1 -> exp ~ 450 overflows e4m3's
# 448). The bias cancels exactly in numerator/denominator.
EXP_BIAS = 2.0

LAST_RESULTS = None  # test harness can read exec_time_ns etc. from here


def _emit(nc, tc, dram):
    xT_d, wqT_d, wkT_d, wvaT_d, cosT2_d, sinT2t_d, projwT_d, outp_d = dram

    with tc.tile_pool(name="persist", bufs=1) as pp:
        qT = [pp.tile([128, N], F16, tag=f"qT{t}", name=f"qT{t}") for t in range(3)]
        kT = [pp.tile([128, N], F16, tag=f"kT{t}", name=f"kT{t}") for t in range(3)]
        # V[nt]: per local head j, cols 128j:128j+64 = 1.0 (denominator rows),
        # cols 128j+64:128j+128 = v values
        if PV_FP8:
            # V8[m]: k-tile pair (2m, 2m+1); plane j at cols [768j : 768j+768],
            # within a plane the per-head 128-col block is [ones 64 | vals 64]
            V = [
                pp.tile([128, 2 * 2 * DL], F8, tag=f"V{t}", name=f"V{t}")
                for t in range(NT // 2)
            ]
        else:
            V = [
                pp.tile([128, 2 * DL], F16, tag=f"V{t}", name=f"V{t}")
                for t in range(NT)
            ]
        attnT = [pp.tile([128, N], F16, tag=f"aT{t}", name=f"aT{t}") for t in range(3)]

        # 0/1 causal mask for the diagonal 128x128 block: 1 where q >= k
        mask01 = pp.tile([128, 256], F16, tag="mask01", name="mask01")
        ebias = pp.tile([128, 1], F32, tag="ebias", name="ebias")

        with (
            tc.tile_pool(name="mm_ps", bufs=2, space="PSUM") as mmp,
            tc.tile_pool(name="score_ps", bufs=2, space="PSUM") as sp,
            tc.tile_pool(name="out_ps", bufs=2, space="PSUM") as op,
            tc.tile_pool(name="qkv_sb", bufs=1) as wp,
        ):
            wq = [wp.tile([128, DL], F16, tag=f"wq{t}", name=f"wq{t}") for t in range(CT)]
            wk = [wp.tile([128, DL], F16, tag=f"wk{t}", name=f"wk{t}") for t in range(CT)]
            wva = [
                wp.tile([128, DL], F16, tag=f"wva{t}", name=f"wva{t}") for t in range(CT)
            ]
            pw = [wp.tile([128, C], F16, tag=f"pw{t}", name=f"pw{t}") for t in range(3)]
            cosT2 = wp.tile([128, N], F16, tag="cosT2", name="cosT2")
            sinT2t = wp.tile([128, N], F16, tag="sinT2t", name="sinT2t")
            xtbs = {}

            def load_x(nb):
                nsl = slice(QB * nb, QB * (nb + 1))
                xtb = [
                    wp.tile([128, QB], F16, tag="xtb", bufs=12, name=f"xtb{nb}_{t}")
                    for t in range(CT)
                ]
                for t in range(CT):
                    eng = nc.sync if t % 2 == 0 else nc.gpsimd
                    eng.dma_start(xtb[t][:], xT_d[128 * t : 128 * (t + 1), nsl])
                xtbs[nb] = xtb

            def qk_chain(nb, dtile, mat):
                """One q-or-k 6-matmul chain + rope for head-pair dtile of block nb."""
                nsl = slice(QB * nb, QB * (nb + 1))
                xtb = xtbs[nb]
                w, dest = (wq, qT) if mat == "q" else (wk, kT)
                ps = mmp.tile([128, QB], F32, tag="mm", name=f"ps_{mat}{nb}_{dtile}")
                for ct in range(CT):
                    nc.tensor.matmul(
                        ps[:],
                        w[ct][:, 128 * dtile : 128 * (dtile + 1)],
                        xtb[ct][:],
                        start=(ct == 0),
                        stop=(ct == CT - 1),
                    )
                wsin = wp.tile(
                    [128, QB], F16, tag="wsin", bufs=3, name=f"ws_{mat}{nb}_{dtile}"
                )
                dst = dest[dtile][:, nsl]
                if nb == 3:
                    # ACT is exp-saturated while block 3 rides qb=2's fillers:
                    # keep its rope off the ACT queue entirely
                    qraw = None
                    nc.vector.scalar_tensor_tensor(
                        out=wsin[:], in0=ps[:], scalar=1.0, in1=sinT2t[:, nsl],
                        op0=ALU.mult, op1=ALU.mult,
                    )
                else:
                    # ACT evacuates PSUM (it has slack vs exp), DVE runs fp16 2x
                    qraw = wp.tile(
                        [128, QB], F16, tag="qraw", bufs=3,
                        name=f"qr_{mat}{nb}_{dtile}",
                    )
                    nc.scalar.copy(qraw[:], ps[:])
                    nc.vector.tensor_mul(wsin[:], qraw[:], sinT2t[:, nsl])
                wrot = wp.tile(
                    [128, QB], F16, tag="wrot", bufs=3, name=f"wr_{mat}{nb}_{dtile}"
                )
                # swap 32-row blocks within each 64: (0,1,2,3)->(1,0,3,2)
                for blk in range(4):
                    lo = 32 * blk
                    swp = 32 * (blk + 1) if blk % 2 == 0 else 32 * (blk - 1)
                    nc.sync.dma_start(wrot[lo : lo + 32, :], wsin[swp : swp + 32, :])
                if nb == 3:
                    nc.vector.scalar_tensor_tensor(
                        out=dst, in0=ps[:], scalar=1.0, in1=cosT2[:, nsl],
                        op0=ALU.mult, op1=ALU.mult,
                    )
                else:
                    nc.vector.tensor_mul(dst, qraw[:], cosT2[:, nsl])
                nc.vector.tensor_add(dst, dst, wrot[:])

            def v_sub(nb, sub):
                nt = 4 * nb + sub
                ps = mmp.tile([128, DL], F32, tag="mm", name=f"ps_v{nt}")
                for ct in range(CT):
                    nc.tensor.matmul(
                        ps[:],
                        xtbs[nb][ct][:, 128 * sub : 128 * (sub + 1)],
                        wva[ct][:],
                        start=(ct == 0),
                        stop=(ct == CT - 1),
                    )
                if PV_FP8:
                    v_dst = V[nt // 2][:].rearrange(
                        "p (j h c) -> p j h c", j=2, c=128
                    )[:, nt % 2, :, HD:128]
                else:
                    v_dst = V[nt][:].rearrange("p (h c) -> p h c", c=128)[
                        :, :, HD:128
                    ]
                nc.vector.tensor_copy(v_dst, ps[:])

            def proj_tile(nt, tail=False):
                nsl = slice(128 * nt, 128 * (nt + 1))
                osb = wp.tile([128, C], F16, tag="osb", bufs=2, name=f"osb{nt}")
                for half in range(2):
                    ps = mmp.tile([128, 384], F32, tag="mm", name=f"pj{half}_{nt}")
                    for ct in range(3):
                        nc.tensor.matmul(
                            ps[:],
                            attnT[ct][:, nsl],
                            pw[ct][:, 384 * half : 384 * (half + 1)],
                            start=(ct == 0),
                            stop=(ct == 2),
                            skip_group_check=True,
                        )
                    if tail:  # ACT is exp-free at the very end
                        nc.scalar.copy(osb[:, 384 * half : 384 * (half + 1)], ps[:])
                        eng = (nc.gpsimd, nc.sync, nc.scalar)[(2 * nt + half) % 3]
                    else:
                        nc.vector.tensor_copy(
                            osb[:, 384 * half : 384 * (half + 1)], ps[:]
                        )
                        eng = nc.gpsimd if half == 0 else nc.sync
                    eng.dma_start(
                        outp_d[nsl, 384 * half : 384 * (half + 1)],
                        osb[:, 384 * half : 384 * (half + 1)],
                    )

            # ---------------- fused pipeline ----------------
            # startup loads: ACT queue stays compute-only (exps + rope evacs);
            # everything rides the SP and GPSIMD queues in need-order.
            for t in range(CT):
                nc.scalar.dma_start(wq[t][:], wqT_d[128 * t : 128 * (t + 1), :])
            load_x(0)
            for t in range(CT):
                nc.gpsimd.dma_start(wk[t][:], wkT_d[128 * t : 128 * (t + 1), :])
            nc.sync.dma_start(cosT2[:], cosT2_d[:])
            nc.sync.dma_start(sinT2t[:], sinT2t_d[:])
            for t in range(CT):
                nc.gpsimd.dma_start(wva[t][:], wvaT_d[128 * t : 128 * (t + 1), :])
            load_x(1)
            for t in range(3):
                nc.gpsimd.dma_start(pw[t][:], projwT_d[128 * t : 128 * (t + 1), :])
            # memsets go behind the gpsimd DMA triggers so loads fire first
            nc.gpsimd.memset(ebias[:], -EXP_BIAS)
            nc.gpsimd.memset(mask01[:], 1.0)
            nc.gpsimd.affine_select(
                out=mask01[:],
                in_=mask01[:],
                compare_op=ALU.is_ge,
                fill=0.0,
                base=0,
                pattern=[[0, 2], [1, 128]],
                channel_multiplier=-1,
            )
            if PV_FP8:
                for m in range(NT // 2):
                    ones_cols = V[m][:].rearrange(
                        "p (j h c) -> p j h c", j=2, c=128
                    )[:, :, :, 0:HD]
                    nc.gpsimd.memset(ones_cols, 1.0)
            else:
                for nt in range(NT):
                    ones_cols = V[nt][:].rearrange("p (h c) -> p h c", c=128)[
                        :, :, 0:HD
                    ]
                    nc.gpsimd.memset(ones_cols, 1.0)
            # dense prologue: qkv block 0
            for d in range(3):
                qk_chain(0, d, "q")
                qk_chain(0, d, "k")
            for s in range(4):
                v_sub(0, s)

            # filler chains woven into the attention loop of each query block
            def fillers_for(qb):
                fl = []
                nb = qb + 1
                if nb < NQB:
                    if nb + 1 < NQB:
                        fl.append(lambda nb=nb: load_x(nb + 1))
                    for d in range(3):
                        fl.append(lambda d=d, nb=nb: qk_chain(nb, d, "q"))
                        fl.append(lambda d=d, nb=nb: qk_chain(nb, d, "k"))
                    for s in range(4):
                        fl.append(lambda s=s, nb=nb: v_sub(nb, s))
                if qb >= 1:
                    for nt in range(4 * (qb - 1), 4 * (qb - 1) + 4):
                        fl.append(lambda nt=nt: proj_tile(nt))
                return fl

            ap = pp
            for qb in range(NQB):
                fl = fillers_for(qb)
                fi = 0
                nkt = 4 * qb + 4
                ngroups = 3 * (nkt // 2)
                gidx = 0
                for pt in range(3):
                    qs = QB * qb
                    psX = [
                        op.tile([128, QB], F32, tag="outps", name=f"ps{hh}_{pt}_{qb}")
                        for hh in range(2)
                    ]
                    pends = []  # pipelined pair-group descriptors
                    depth = 1
                    for ktp in range(0, nkt, 2):
                        kts = (ktp, ktp + 1)
                        a0s, Ss = {}, {}
                        for kt in kts:
                            a = 128 * kt - QB * qb
                            a0 = max(a, 0)
                            a0s[kt] = a0
                            ksl = slice(128 * kt, 128 * (kt + 1))
                            S = sp.tile(
                                [128, 2 * QB], F32, tag="sc", name=f"S{pt}_{qb}_{kt}"
                            )
                            Ss[kt] = S
                            for hh in range(2):
                                prow = slice(64 * hh, 64 * hh + 64)
                                nc.tensor.matmul(
                                    S[:, QB * hh + a0 : QB * (hh + 1)],
                                    kT[pt][prow, ksl],
                                    qT[pt][prow, qs + a0 : qs + QB],
                                    start=True,
                                    stop=True,
                                )
                        # filler: keep PE busy while exp of this group runs
                        gidx += 1
                        while fi < len(fl) and fi * ngroups < len(fl) * gidx:
                            fl[fi]()
                            fi += 1
                        diag = 128 * kts[0] >= QB * qb
                        if PV_FP8:
                            P8 = ap.tile(
                                [128, 2 * 2 * QB], F8, tag="probs", bufs=4,
                                name=f"P{pt}_{qb}_{ktp}",
                            )
                            p4 = P8[:].rearrange(
                                "p (j g q) -> p j g q", j=2, q=QB
                            )
                            if diag:
                                # plane 1's fully-masked 128-col zone (between
                                # the pair's base offset and its own diagonal)
                                nc.gpsimd.memset(
                                    p4[:, 1, :, a0s[kts[0]] : a0s[kts[0]] + 128], 0.0
                                )
                            for j, kt in enumerate(kts):
                                a0, S = a0s[kt], Ss[kt]
                                s2 = S[:].rearrange("p (g q) -> p g q", q=QB)[
                                    :, :, a0:QB
                                ]
                                nc.scalar.activation(
                                    p4[:, j, :, a0:QB], s2, AF.Exp,
                                    bias=ebias[:], scale=SCALE,
                                )
                                if diag:
                                    for hh in range(2):
                                        nc.vector.tensor_mul(
                                            p4[:, j, hh, a0 : a0 + 128],
                                            p4[:, j, hh, a0 : a0 + 128],
                                            mask01[:],
                                        )
                            this = ("f8", ktp, a0s[kts[0]], P8)
                        else:
                            Ps = {}
                            for kt in kts:
                                a0, S = a0s[kt], Ss[kt]
                                P = ap.tile(
                                    [128, 2 * QB], F16, tag="probs", bufs=8,
                                    name=f"P{pt}_{qb}_{kt}",
                                )
                                Ps[kt] = P
                                s2 = S[:].rearrange("p (g q) -> p g q", q=QB)[
                                    :, :, a0:QB
                                ]
                                p2 = P[:].rearrange("p (g q) -> p g q", q=QB)[
                                    :, :, a0:QB
                                ]
                                nc.scalar.activation(p2, s2, AF.Exp, scale=SCALE)
                                if 128 * kt >= QB * qb:  # diagonal tile
                                    pm = P[:].rearrange(
                                        "p (g q) -> p g q", q=QB
                                    )[:, :, a0 : a0 + 128]
                                    m2 = mask01[:].rearrange(
                                        "p (g q) -> p g q", q=128
                                    )
                                    nc.vector.tensor_mul(pm, pm, m2)
                            this = ("f16", kts, a0s, Ps)
                        pends.append(this)
                        if len(pends) > depth:
                            _emit_pv(nc, pends.pop(0), psX, V, pt, nkt)
                    for pend in pends:
                        _emit_pv(nc, pend, psX, V, pt, nkt)
                    last_pt = qb == NQB - 1 and pt == 2
                    # normalize: psX rows 0:64 = den (replicated by the ones
                    # columns of V), rows 64:128 = values. head B (hh=1) writes
                    # attnT rows 64:128 directly; head A via staging + shift DMA.
                    # On the final head pair, work in 128-column chunks with the
                    # idle ACT queue carrying the DMAs and emit a proj tile as
                    # soon as its token range is normalized.
                    dq = nc.scalar if last_pt else nc.sync
                    rr = [
                        ap.tile([128, QB], F32, tag="r", bufs=2, name=f"r{hh}_{pt}_{qb}")
                        for hh in range(2)
                    ]
                    tb = ap.tile([128, QB], F16, tag="tmpB", bufs=2, name=f"tB{pt}_{qb}")
                    for hh in range(2):
                        nc.vector.reciprocal_approx_fast(
                            rr[hh][0:64, :], psX[hh][0:64, :]
                        )
                    nchunk = 2 if last_pt else 1
                    cw = QB // nchunk
                    for ci in range(nchunk):
                        cs = slice(ci * cw, (ci + 1) * cw)
                        osl = slice(qs + ci * cw, qs + (ci + 1) * cw)
                        for hh in range(2):
                            dq.dma_start(rr[hh][64:128, cs], rr[hh][0:64, cs])
                        nc.vector.scalar_tensor_tensor(
                            out=tb[64:128, cs],
                            in0=psX[0][64:128, cs],
                            scalar=1.0,
                            in1=rr[0][64:128, cs],
                            op0=ALU.mult,
                            op1=ALU.mult,
                        )
                        dq.dma_start(attnT[pt][0:64, osl], tb[64:128, cs])
                        nc.vector.scalar_tensor_tensor(
                            out=attnT[pt][64:128, osl],
                            in0=psX[1][64:128, cs],
                            scalar=1.0,
                            in1=rr[1][64:128, cs],
                            op0=ALU.mult,
                            op1=ALU.mult,
                        )
                        if last_pt:
                            for nt in range(4 * qb + 2 * ci, 4 * qb + 2 * ci + 2):
                                proj_tile(nt, tail=True)
                while fi < len(fl):
                    fl[fi]()
                    fi += 1


def _emit_pv(nc, pend, psX, V, pt, nkt):
    if pend[0] == "f8":
        _, ktp, a0, P8 = pend
        first, last = ktp == 0, ktp + 2 >= nkt
        v4 = V[ktp // 2][:].rearrange("p (j x) -> p j x", x=2 * DL)
        p4 = P8[:].rearrange("p (j x) -> p j x", x=2 * QB)
        for hh in range(2):
            h = 2 * pt + hh
            nc.tensor.matmul(
                psX[hh][:, a0:QB],
                v4[:, :, 128 * h : 128 * (h + 1)],
                p4[:, :, QB * hh + a0 : QB * (hh + 1)],
                start=first,
                stop=last,
                skip_group_check=True,
                perf_mode=mybir.MatmulPerfMode.DoubleRow,
            )
        return
    _, kts, a0s, Ps = pend
    for kt in kts:
        a0, P = a0s[kt], Ps[kt]
        first, last = kt == 0, kt == nkt - 1
        for hh in range(2):
            h = 2 * pt + hh
            nc.tensor.matmul(
                psX[hh][:, a0:QB],
                V[kt][:, 128 * h : 128 * (h + 1)],
                P[:, QB * hh + a0 : QB * (hh + 1)],
                start=first,
                stop=last,
                skip_group_check=True,
            )


def _build_program():
    nc = bacc.Bacc(
        "TRN2",
        target_bir_lowering=False,
        debug=False,
        num_devices=NCORES,
    )

    dram = (
        nc.dram_tensor("xT", [C, N], F16, kind="ExternalInput"),
        nc.dram_tensor("wqT", [C, DL], F16, kind="ExternalInput"),
        nc.dram_tensor("wkT", [C, DL], F16, kind="ExternalInput"),
        nc.dram_tensor("wvaT", [C, DL], F16, kind="ExternalInput"),
        nc.dram_tensor("cosT2", [128, N], F16, kind="ExternalInput"),
        nc.dram_tensor("sinT2t", [128, N], F16, kind="ExternalInput"),
        nc.dram_tensor("projwT", [DL, C], F16, kind="ExternalInput"),
        nc.dram_tensor("outp", [N, C], F16, kind="ExternalOutput"),
    )

    with tile.TileContext(nc) as tc:
        _emit(nc, tc, dram)

    nc.compile()
    return nc


def _rope_tables():
    # mirror reference.rope_tables in float32 (keep the f32 product!)
    inv_freq = 1.0 / np.power(
        np.float32(10000.0), np.arange(0, HD, 2, dtype=np.float32) / np.float32(HD)
    )
    t = np.arange(N, dtype=np.float32)
    freqs = (t[:, None] * inv_freq[None, :].astype(np.float32)).astype(np.float32)
    emb = np.concatenate([freqs, freqs], axis=-1)  # [N, 64]
    return np.cos(emb).astype(np.float32), np.sin(emb).astype(np.float32)


def _make_in_maps(x, qkv_w, proj_w):
    f16 = np.float16
    cos, sin = _rope_tables()  # [N, 64]
    dd = np.arange(128) % HD
    cosT2 = np.ascontiguousarray(cos.T[dd, :]).astype(f16)  # [128, N]
    # sin_tau sign such that q' = q*cos + swap32(q*sin_tau)
    sgn = np.where((dd % HD) < (HD // 2), np.float32(1.0), np.float32(-1.0))
    sinT2t = np.ascontiguousarray(sin.T[dd, :] * sgn[:, None]).astype(f16)

    in_maps = []
    for core in range(NCORES):
        b, g = core // G, core % G
        heads = [g * HL + j for j in range(HL)]
        cols = np.concatenate([np.arange(HD * h, HD * h + HD) for h in heads])
        xT = np.ascontiguousarray(x[b].T).astype(f16)
        wqT = np.ascontiguousarray(qkv_w[cols, :].T).astype(f16)
        wkT = np.ascontiguousarray(qkv_w[C + cols, :].T).astype(f16)
        wvaT = np.ascontiguousarray(qkv_w[2 * C + cols, :].T).astype(f16)
        projwT = np.ascontiguousarray(proj_w[:, cols].T).astype(f16)
        in_maps.append(
            {
                "xT": xT,
                "wqT": wqT,
                "wkT": wkT,
                "wvaT": wvaT,
                "cosT2": cosT2,
                "sinT2t": sinT2t,
                "projwT": projwT,
            }
        )
    return in_maps


def _install_ntff_hook():
    """Wire the axon NTFF profiling hook if the image's antenv lacks it."""
    import types

    try:
        from antenv.axon_hooks import get_axon_ntff_profile_hook  # noqa: F401

        return True
    except ImportError:
        pass
    try:
        import antenv
        from trn_agent_boot.trn_boot import _ntff_profile_via_ctypes

        hook = _ntff_profile_via_ctypes("/opt/axon/libaxon_pjrt.so")
        mod = types.ModuleType("antenv.axon_hooks")
        holder = {"hook": hook}
        mod.set_axon_ntff_profile_hook = lambda h: holder.__setitem__("hook", h)
        mod.get_axon_ntff_profile_hook = lambda: holder["hook"]
        sys.modules["antenv.axon_hooks"] = mod
        antenv.axon_hooks = mod
        return hook is not None
    except Exception as e:  # pragma: no cover
        print(f"ntff hook install failed: {e}")
        return False


_PROGRAM = None


def kernel(x, qkv_w, proj_w, proj_b):
    global _PROGRAM, LAST_RESULTS
    x = np.asarray(x, dtype=np.float32)
    qkv_w = np.asarray(qkv_w, dtype=np.float32)
    proj_w = np.asarray(proj_w, dtype=np.float32)
    proj_b = np.asarray(proj_b, dtype=np.float32)

    if _PROGRAM is None:
        _PROGRAM = _build_program()
    nc = _PROGRAM

    in_maps = _make_in_maps(x, qkv_w, proj_w)
    trace = bool(int(os.environ.get("KERNEL_TRACE", "0")))
    if trace:
        trace = _install_ntff_hook()
    res = run_bass_kernel_spmd(nc, in_maps, list(range(NCORES)), trace=trace)
    LAST_RESULTS = res

    out = np.empty((B, N, C), dtype=np.float32)
    for b in range(B):
        out[b] = res.results[G * b]["outp"].astype(np.float32) + res.results[
            G * b + 1
        ]["outp"].astype(np.float32)
    out += proj_b[None, None, :]
    return out


if __name__ == "__main__":
    x = np.random.randn(B, N, C).astype(np.float32)
    qkv_w = np.random.randn(3 * C, C).astype(np.float32)
    proj_w = np.random.randn(C, C).astype(np.float32)
    maps = _make_in_maps(x, qkv_w, proj_w)
    for k, v in maps[0].items():
        print(k, v.shape, v.dtype)
